# revision 19
# baseline (speedup 1.0000x reference)
"""Trainium2 Bass kernel for nn_Attention_40991167873617 (sparse_attention).

Computation (reference):
    ep    = x[:,0] * x[:,1]                          # [B, E]
    trees = x[:,2:]                                  # [B, T, E]
    h     = relu(cat([ep, trees], -1) @ attn_w + b)  # [B, T, A]
    l     = h @ proj_w (+ proj_b)                    # [B, T, 1]
    s     = softmax(l, axis=1)
    out   = sum(s * trees, 1) / T                    # [B, E]
    returns (out, ep)

v4 strategy (867us baseline -> 479us):
  - Pure data-parallel over 8 cores (B/8 = 1024 rows each).
  - treesT [E, rows] bf16 host-transposed; main hT matmul = W2T k-tiles
    + 2-way row-group-packed one-hot ep@W1(+b) fold MMs; logits trail one
    2-block group and the previous chunk's weighted sum trails the whole
    h-phase so the PE stream stays dense (HAM warmth).
  - Logits: proj_w placed in column j of an M=64 stationary so block r's
    logits land on PSUM PARTITION {r or 64+r-8} -> per 16-block chunk ONE
    [128,512] psum tile holds all 16 blocks' logits. exp = 2 ACT
    instructions per chunk (vs 128x [1,512]); Z = one DVE reduce.
  - Broadcast w-row -> 128 partitions via PE selector matmul (K=16
    one-hot-row stationary), PSUM -> SBUF copy split ACT/DVE.
  - Weighted tree sum: DVE/GPSIMD multiply (bf16 2x), bf16 fold tree
    (t: 64->32->16->8) + final segmented reduce on DVE.
  - Epilogue: PE transpose [E, b] -> [b, E], scale by 1/(T*Z).
"""

import sys

sys.path.insert(0, "/opt/trn_rl_repo")

from contextlib import ExitStack

import ml_dtypes
import numpy as np

BF16NP = ml_dtypes.bfloat16

import concourse.bacc as bacc
import concourse.tile as tile
from concourse import mybir
from concourse.alu_op_type import AluOpType
from concourse.bass_utils import run_bass_kernel_spmd

AF = mybir.ActivationFunctionType
AX = mybir.AxisListType
F32 = mybir.dt.float32
BF16 = mybir.dt.bfloat16

B, T, E, A = 8192, 64, 256, 256
NCORES = 8
BC = B // NCORES          # 1024 batch rows per core
ROWS = BC * T             # 65536 rows per core
RB = 512                  # rows per block
BPB = RB // T             # 8 batch rows per block
NBLK = ROWS // RB         # 128 blocks per core
CHUNK_BLKS = 16           # blocks per chunk
NCHUNK = NBLK // CHUNK_BLKS   # 8 chunks
CB = CHUNK_BLKS * BPB     # 128 batch rows per chunk
CROWS = CHUNK_BLKS * RB   # 8192 rows per chunk
QB = 4                    # blocks per quad (mult/fold granularity)
NQ = CHUNK_BLKS // QB     # 4 quads per chunk
QROWS = QB * RB           # 2048

PROFILE = False
LAST_EXEC_NS = None
LAST_RESULTS = None

_CACHE = {}


def _body(ctx, tc, ins, outs):
    nc = tc.nc
    (tT_d, x01_d, aw_d, aw1_d, ab_d, abb_d, pbig_d, selr_d, oh_d, id_d) = ins
    oa_d, oe_d = outs

    consts = ctx.enter_context(tc.tile_pool(name="consts", bufs=1))

    # --- load constants -------------------------------------------------
    wsb = consts.tile([128, 4 * A], BF16, tag="wsb")       # attn_w k-tiles
    for k in range(4):
        nc.sync.dma_start(wsb[:, k * A:(k + 1) * A], aw_d[k * 128:(k + 1) * 128, :])
    absb = consts.tile([128, 2], F32, tag="absb")
    for at in range(2):
        nc.sync.dma_start(absb[:, at:at + 1], ab_d[at * 128:(at + 1) * 128, :])
    pbig = consts.tile([128, 16 * 64], BF16, tag="pbig")   # logits stationaries
    nc.sync.dma_start(pbig[:], pbig_d[:])
    selr = consts.tile([64, 16 * 128], BF16, tag="selr")   # broadcast selectors
    nc.sync.dma_start(selr[:], selr_d[:])
    ohsb = consts.tile([64, 4 * RB], BF16, tag="ohsb")     # 2 row-replicas
    nc.sync.dma_start(ohsb[:], oh_d[:])
    uT32 = consts.tile([64, 32 * A], BF16, tag="uT32")     # [64, (grp, A)] 2 replicas
    idsb = consts.tile([128, 128], F32, tag="idsb")
    nc.sync.dma_start(idsb[:], id_d[:])

    zAll = consts.tile([64, 8 * NCHUNK], F32, tag="zAll")  # Z: rows {0-7, 32-39}
    rzall = consts.tile([128, 8], F32, tag="rzall")        # 1/(T*Z) per b
    oTacc = consts.tile([128, 2 * BC], F32, tag="oTacc")   # [128,(et,b)]

    # --- prologue: epT, element_product output, uT = ep @ W1 ------------
    with tc.tile_pool(name="prol_c", bufs=1) as prol_c, \
         tc.tile_pool(name="prol_ps", bufs=2, space="PSUM") as prol_ps, \
         tc.tile_pool(name="prol_sb", bufs=2) as prol_sb:
        abbc = prol_c.tile([128, A], F32, tag="abbc")      # attn_b replicated [128, A]
        nc.sync.dma_start(abbc[:], abb_d[:])
        wsb1 = prol_c.tile([128, 2 * A], F32, tag="wsb1")  # f32 W1 for uT matmul
        for k in range(2):
            nc.sync.dma_start(wsb1[:, k * A:(k + 1) * A], aw1_d[k * 128:(k + 1) * 128, :])
        x01sb = prol_c.tile([128, 2 * 2 * BC], F32, tag="x01sb")  # [128,(et,{x0,x1},b)]
        for et in range(2):
            nc.sync.dma_start(x01sb[:, et * 2 * BC:(et + 1) * 2 * BC],
                              x01_d[et * 128:(et + 1) * 128, :])
        epTsb = prol_c.tile([128, 2 * BC], F32, tag="epTsb")      # [128,(et,b)]
        uTsb = prol_c.tile([128, A], BF16, tag="uTsb")
        for et in range(2):
            nc.vector.tensor_tensor(
                epTsb[:, et * BC:(et + 1) * BC],
                x01sb[:, et * 2 * BC:et * 2 * BC + BC],
                x01sb[:, et * 2 * BC + BC:et * 2 * BC + 2 * BC],
                op=AluOpType.mult,
            )
        for bt in range(8):
            # element_product natural layout via PE transpose
            epn = prol_sb.tile([128, E], F32, tag="epn")
            for et in range(2):
                pt = prol_ps.tile([128, 128], F32, tag="ept")
                nc.tensor.transpose(pt[:], epTsb[:, et * BC + bt * 128:et * BC + (bt + 1) * 128], idsb[:])
                nc.scalar.copy(epn[:, et * 128:(et + 1) * 128], pt[:])
            nc.sync.dma_start(oe_d[bt * 128:(bt + 1) * 128, :], epn[:])
            # uT tile: ep @ W1  -> [b, A]
            ups = prol_ps.tile([128, A], F32, tag="ups")
            for kt in range(2):
                nc.tensor.matmul(
                    ups[:],
                    epTsb[:, kt * BC + bt * 128:kt * BC + (bt + 1) * 128],
                    wsb1[:, kt * A:(kt + 1) * A],
                    start=(kt == 0), stop=(kt == 1),
                )
            nc.vector.tensor_tensor(uTsb[:], ups[:], abbc[:], op=AluOpType.add)
            for q in range(4):
                for rep in range(2):
                    nc.scalar.dma_start(
                        uT32[32 * rep:32 * (rep + 1),
                             (bt * 4 + q) * A:(bt * 4 + q + 1) * A],
                        uTsb[32 * q:32 * (q + 1), :])

    # --- main pipeline ---------------------------------------------------
    ttp = ctx.enter_context(tc.tile_pool(name="ttp", bufs=3))      # trees chunk tiles
    sbp = ctx.enter_context(tc.tile_pool(name="sbp", bufs=6))      # htsb / wbcs
    mp = ctx.enter_context(tc.tile_pool(name="mp", bufs=2))        # m quad tiles
    fp = ctx.enter_context(tc.tile_pool(name="fp", bufs=2))        # fold tiles
    smp = ctx.enter_context(tc.tile_pool(name="smp", bufs=2))      # wexp
    _ps_stack = ExitStack()
    htps = _ps_stack.enter_context(tc.tile_pool(name="htps", bufs=2, space="PSUM"))
    lgps = _ps_stack.enter_context(tc.tile_pool(name="lgps", bufs=2, space="PSUM"))
    wps = _ps_stack.enter_context(tc.tile_pool(name="wps", bufs=2, space="PSUM"))

    def emit_wsum(tt0, tt1, wexp, ch):
        for q in range(NQ):
            mq = mp.tile([128, 2 * QROWS], BF16, tag="mq")
            for rq in range(QB):
                r = q * QB + rq
                off = rq * RB
                # broadcast row of wexp to 128 partitions (PE selector MM)
                wbc = wps.tile([128, RB], F32, tag="wbc")
                nc.tensor.matmul(wbc[:], selr[:, r * 128:(r + 1) * 128], wexp[:],
                                 start=True, stop=True)
                wbcs = sbp.tile([128, RB], BF16, tag="wbcs")
                if r % 2 == 0:
                    nc.scalar.copy(wbcs[:], wbc[:])
                else:
                    nc.vector.tensor_copy(wbcs[:], wbc[:])
                nc.vector.tensor_tensor(
                    mq[:, off:off + RB],
                    tt0[:, r * RB:(r + 1) * RB], wbcs[:], op=AluOpType.mult)
                nc.gpsimd.tensor_tensor(
                    mq[:, QROWS + off:QROWS + off + RB],
                    tt1[:, r * RB:(r + 1) * RB], wbcs[:], op=AluOpType.mult)
            # fold tree over t: 64 -> 32 -> 16 -> 8, then segmented reduce
            for et in range(2):
                src = mq[:, et * QROWS:(et + 1) * QROWS]
                sv = src.rearrange("p (s t) -> p s t", t=T)
                f1 = fp.tile([128, QROWS // 2], BF16, tag="f1")
                nc.vector.tensor_tensor(
                    f1[:].rearrange("p (s t) -> p s t", t=32),
                    sv[:, :, 0:32], sv[:, :, 32:64], op=AluOpType.add)
                f1v = f1[:].rearrange("p (s t) -> p s t", t=32)
                f2 = fp.tile([128, QROWS // 4], BF16, tag="f2")
                nc.vector.tensor_tensor(
                    f2[:].rearrange("p (s t) -> p s t", t=16),
                    f1v[:, :, 0:16], f1v[:, :, 16:32], op=AluOpType.add)
                f2v = f2[:].rearrange("p (s t) -> p s t", t=16)
                f3 = fp.tile([128, QROWS // 8], BF16, tag="f3")
                nc.vector.tensor_tensor(
                    f3[:].rearrange("p (s t) -> p s t", t=8),
                    f2v[:, :, 0:8], f2v[:, :, 8:16], op=AluOpType.add)
                nc.vector.tensor_reduce(
                    oTacc[:, et * BC + ch * CB + q * QB * BPB:
                          et * BC + ch * CB + (q + 1) * QB * BPB],
                    f3[:].rearrange("p (s t) -> p s t", t=8),
                    axis=AX.X, op=AluOpType.add)

    pend = None
    for ch in range(NCHUNK):
        tt0 = ttp.tile([128, CROWS], BF16, tag="tt0")
        tt1 = ttp.tile([128, CROWS], BF16, tag="tt1")
        nc.sync.dma_start(tt0[:], tT_d[0:128, ch * CROWS:(ch + 1) * CROWS])
        nc.sync.dma_start(tt1[:], tT_d[128:256, ch * CROWS:(ch + 1) * CROWS])

        lg = lgps.tile([128, RB], F32, tag="lg")
        hts = {}
        # 2-block groups: w-MMs, then 2-way row-packed one-hot u-MMs
        # (tile_position row groups 0/32 run concurrently), then relu.
        # Logits trail one group so the PE never stalls on a fresh relu.
        for grp in range(CHUNK_BLKS // 2 + 1):
            if grp < CHUNK_BLKS // 2:
                h2 = {}
                for r in (2 * grp, 2 * grp + 1):
                    ht2 = htps.tile([128, 2 * RB], F32, tag="ht2")
                    for at in range(2):
                        hs = ht2[:, at * RB:(at + 1) * RB]
                        nc.tensor.matmul(hs, wsb[:, 2 * A + at * 128:2 * A + at * 128 + 128],
                                         tt0[:, r * RB:(r + 1) * RB], start=True, stop=False)
                        nc.tensor.matmul(hs, wsb[:, 3 * A + at * 128:3 * A + at * 128 + 128],
                                         tt1[:, r * RB:(r + 1) * RB], start=False, stop=False)
                    h2[r] = ht2
                for at in range(2):
                    for jj, r in enumerate((2 * grp, 2 * grp + 1)):
                        g = ch * CHUNK_BLKS + r
                        b0 = 32 * jj
                        nc.tensor.matmul(
                            h2[r][:, at * RB:(at + 1) * RB],
                            uT32[b0:b0 + 32, (g // 4) * A + at * 128:(g // 4) * A + at * 128 + 128],
                            ohsb[b0:b0 + 32, (g % 4) * RB:(g % 4 + 1) * RB],
                            start=False, stop=True)
                for r in (2 * grp, 2 * grp + 1):
                    htsb = sbp.tile([128, 2 * RB], BF16, tag="htsb")
                    nc.scalar.activation(htsb[:], h2[r][:], AF.Relu)
                    hts[r] = htsb
            if grp >= 1:
                for r in (2 * grp - 2, 2 * grp - 1):
                    htsb = hts.pop(r)
                    base, j = (0, r) if r < 8 else (64, r - 8)
                    for at in range(2):
                        nc.tensor.matmul(
                            lg[base:base + 64, :],
                            pbig[:, (at * 8 + j) * 64:(at * 8 + j + 1) * 64],
                            htsb[:, at * RB:(at + 1) * RB],
                            start=(j == 0 and at == 0), stop=(j == 7 and at == 1))

        # --- exp: 2 ACT instructions for the whole chunk ---
        wexp = smp.tile([64, RB], BF16, tag="wexp")
        nc.vector.memset(wexp[:], 0.0)
        nc.scalar.activation(wexp[0:8, :], lg[0:8, :], AF.Exp)
        nc.scalar.activation(wexp[32:40, :], lg[64:72, :], AF.Exp)
        # --- Z: DVE reduces [8, (8b, 64t)] -> [8, 8] ---
        nc.vector.tensor_reduce(
            zAll[0:8, ch * 8:(ch + 1) * 8],
            wexp[0:8, :].rearrange("p (b t) -> p b t", t=T),
            axis=AX.X, op=AluOpType.add)
        nc.vector.tensor_reduce(
            zAll[32:40, ch * 8:(ch + 1) * 8],
            wexp[32:40, :].rearrange("p (b t) -> p b t", t=T),
            axis=AX.X, op=AluOpType.add)

        # weighted tree sum of the PREVIOUS chunk (PE: its broadcast MMs no
        # longer wait on this chunk's exp; DVE folds overlap next h-phase)
        if pend is not None:
            emit_wsum(*pend)
        pend = (tt0, tt1, wexp, ch)
    emit_wsum(*pend)

    _ps_stack.close()

    # rz: zAll [16, (ch, bb)] -> rzall [128, 8]; rzall[p, ch] = 1/(T*Z[b])
    nc.vector.reciprocal(zAll[0:8, :], zAll[0:8, :])
    nc.vector.reciprocal(zAll[32:40, :], zAll[32:40, :])
    for b0 in (0, 32):
        nc.vector.tensor_scalar(out=zAll[b0:b0 + 8, :], in0=zAll[b0:b0 + 8, :],
                                scalar1=1.0 / T, scalar2=None, op0=AluOpType.mult)
    for ch in range(NCHUNK):
        nc.sync.dma_start(rzall[0:64, ch:ch + 1], zAll[0:8, ch * 8:(ch + 1) * 8])
        nc.sync.dma_start(rzall[64:128, ch:ch + 1], zAll[32:40, ch * 8:(ch + 1) * 8])

    # --- epilogue: transpose [E, b] -> [b, E], scale by 1/(T*Z) ----------
    with tc.tile_pool(name="epi_ps", bufs=2, space="PSUM") as epi_ps, \
         tc.tile_pool(name="epi_sb", bufs=2) as epi_sb:
        for bt in range(8):
            oasb = epi_sb.tile([128, E], F32, tag="oasb")
            for et in range(2):
                pt = epi_ps.tile([128, 128], F32, tag="opt")
                nc.tensor.transpose(pt[:], oTacc[:, et * BC + bt * 128:et * BC + (bt + 1) * 128], idsb[:])
                nc.vector.tensor_scalar(out=oasb[:, et * 128:(et + 1) * 128], in0=pt[:],
                                        scalar1=rzall[:, bt:bt + 1], scalar2=None,
                                        op0=AluOpType.mult)
            nc.sync.dma_start(oa_d[bt * 128:(bt + 1) * 128, :], oasb[:])


def build():
    if "nc" in _CACHE:
        return _CACHE["nc"]
    nc = bacc.Bacc("TRN2", target_bir_lowering=False, debug=False)
    ins = [
        nc.dram_tensor("treesT", [E, ROWS], BF16, kind="ExternalInput").ap(),
        nc.dram_tensor("x01T", [E, 2 * BC], F32, kind="ExternalInput").ap(),
        nc.dram_tensor("attn_w", [2 * E, A], BF16, kind="ExternalInput").ap(),
        nc.dram_tensor("attn_w1f", [E, A], F32, kind="ExternalInput").ap(),
        nc.dram_tensor("attn_b2", [A, 1], F32, kind="ExternalInput").ap(),
        nc.dram_tensor("attn_bb", [128, A], F32, kind="ExternalInput").ap(),
        nc.dram_tensor("proj_big", [128, 16 * 64], BF16, kind="ExternalInput").ap(),
        nc.dram_tensor("selr", [64, 16 * 128], BF16, kind="ExternalInput").ap(),
        nc.dram_tensor("onehot", [64, 4 * RB], BF16, kind="ExternalInput").ap(),
        nc.dram_tensor("ident", [128, 128], F32, kind="ExternalInput").ap(),
    ]
    outs = [
        nc.dram_tensor("out_attn", [BC, E], F32, kind="ExternalOutput").ap(),
        nc.dram_tensor("out_ep", [BC, E], F32, kind="ExternalOutput").ap(),
    ]
    with tile.TileContext(nc) as tc, ExitStack() as ctx:
        _body(ctx, tc, ins, outs)
    nc.compile()
    _CACHE["nc"] = nc
    return nc


def make_in_maps(x, attn_w, attn_b, proj_w, proj_b):
    x = np.asarray(x, dtype=np.float32)
    oh = np.zeros((64, 4 * RB), BF16NP)
    for v in range(4):
        for jj in range(BPB):
            oh[v * BPB + jj, v * RB + jj * T:v * RB + (jj + 1) * T] = 1.0
    oh[32:64] = oh[0:32]
    aw32 = np.asarray(attn_w, np.float32)
    pw = np.asarray(proj_w, np.float32).reshape(A)
    # proj_big: per (a-half, j): [128, 64] slice with col j = pw[a-half]
    pbig = np.zeros((128, 16 * 64), np.float32)
    for a in range(2):
        for j in range(8):
            pbig[:, (a * 8 + j) * 64 + j] = pw[a * 128:(a + 1) * 128]
    # selr: [64, 16*128]: slice r = [64, 128] one-hot row (r<8: r, else 32+r-8)
    selr = np.zeros((64, 16 * 128), BF16NP)
    for r in range(16):
        selr[r if r < 8 else 32 + r - 8, r * 128:(r + 1) * 128] = 1.0
    consts = {
        "attn_w": np.ascontiguousarray(aw32.astype(BF16NP)),
        "selr": selr,
        "attn_bb": np.ascontiguousarray(np.broadcast_to(
            np.asarray(attn_b, np.float32)[None, :], (128, A))),
        "attn_w1f": np.ascontiguousarray(aw32[:E]),
        "attn_b2": np.ascontiguousarray(np.asarray(attn_b, np.float32).reshape(A, 1)),
        "proj_big": pbig.astype(BF16NP),
        "onehot": oh,
        "ident": np.eye(128, dtype=np.float32),
    }
    in_maps = []
    for c in range(NCORES):
        xs = x[c * BC:(c + 1) * BC]
        treesT = np.ascontiguousarray(xs[:, 2:, :].reshape(ROWS, E).T.astype(BF16NP))
        x01T = np.ascontiguousarray(
            np.concatenate([xs[:, 0, :].T, xs[:, 1, :].T], axis=1))
        in_maps.append({"treesT": treesT, "x01T": x01T, **consts})
    return in_maps


def kernel(x, attn_w, attn_b, proj_w, proj_b):
    global LAST_EXEC_NS, LAST_RESULTS
    nc = build()
    in_maps = make_in_maps(x, attn_w, attn_b, proj_w, proj_b)
    kw = {}
    if PROFILE:
        import shutil
        shutil.rmtree("/tmp/ktrace", ignore_errors=True)
        import os
        os.makedirs("/tmp/ktrace", exist_ok=True)
        kw = dict(trace=True, tmpdir="/tmp/ktrace")
    r = run_bass_kernel_spmd(nc, in_maps, list(range(NCORES)), **kw)
    LAST_EXEC_NS = r.exec_time_ns
    LAST_RESULTS = r
    attn = np.concatenate([r.results[c]["out_attn"] for c in range(NCORES)], axis=0)
    ep = np.concatenate([r.results[c]["out_ep"] for c in range(NCORES)], axis=0)
    return attn, ep
